# revision 8
# baseline (speedup 1.0000x reference)
"""CharRNN Trainium2 kernel.

Reference computation (B=64, L=512, V=96, E=256, H=1024):
    x_lin  = embedding[x] @ W_e                # [B, L, H]
    h_t    = tanh(h_{t-1} @ W_h + x_lin[:, t]) # sequential scan over L
    logits = stack(h_t) @ fc_w.T + fc_b        # [B, L, V]
    returns (logits, final_hidden)

Strategy:
  - Data-parallel: batch sharded 8 ways (8 rows/core); the scan is local
    to each core, weights replicated. No collectives.
  - The input projection is folded algebraically: embedding[x] @ W_e ==
    (embedding @ W_e)[x] == onehot(x) @ embW, so each scan step's
    pre-activation is ONE fused PSUM accumulation group:
        z_t.T[jchunk] = sum_k W_h[k,j].T-chunk MMs + embW-chunk MM
    with h kept transposed ([H, batch] chunks of [128, 8]) so the matmul
    output lands directly in the layout the next step consumes - no
    transposes anywhere in the scan.
  - bf16 weights/activations (FWL weight loads), fp32 PSUM accumulation,
    tanh on the scalar engine straight out of PSUM.
  - Everything is SBUF-resident during the scan (weights, one-hot codes,
    full h history); the final logits GEMM reads the history from SBUF.
"""

import numpy as np
import ml_dtypes

B, L, V, E, H = 64, 512, 96, 256, 1024
NCORES = 8
BC = B // NCORES        # batch per core
KC = H // 128           # hidden chunks
VP = 128                # vocab padded to full partition dim (FWL + matmul)
LB = L * BC             # columns in time-major flattened layout

_BF16 = ml_dtypes.bfloat16

_nc_cache = {}
LAST_RUN = {}  # stash of the most recent BassKernelResults (for test harness)


def _build_nc(n_steps):
    import concourse.bass as bass
    import concourse.mybir as mybir
    import concourse.tile as tile
    from concourse import bacc
    from contextlib import ExitStack

    bf16 = mybir.dt.bfloat16
    f32 = mybir.dt.float32
    Tanh = mybir.ActivationFunctionType.Tanh
    Ident = mybir.ActivationFunctionType.Identity

    cols = n_steps * BC

    nc = bacc.Bacc()
    oh_d = nc.declare_dram_parameter("oh", [VP, cols], bf16, isOutput=False)
    h0_d = nc.declare_dram_parameter("h0", [H, BC], bf16, isOutput=False)
    wh_d = nc.declare_dram_parameter("wh", [KC, 128, H], bf16, isOutput=False)
    ew_d = nc.declare_dram_parameter("ew", [VP, H], bf16, isOutput=False)
    fw_d = nc.declare_dram_parameter("fw", [KC, 128, V], bf16, isOutput=False)
    fb_d = nc.declare_dram_parameter("fb", [V, 1], f32, isOutput=False)
    lg_d = nc.declare_dram_parameter("lg", [V, cols], f32, isOutput=True)
    hf_d = nc.declare_dram_parameter("hf", [H, BC], f32, isOutput=True)

    HL = (n_steps + 1) * BC  # history columns per hidden chunk

    # j-chunk groupings per step: one PSUM tile (bank) + one tanh ACT per
    # split, so the scalar engine issues 3 ACTs/step instead of 8 and the
    # PE never waits on a slipping ACT queue.  PE writes and ACT reads of
    # the same step land in different banks (PE-W + ACT-R same-bank is a
    # HW hazard that would serialize).
    SPLITS = [(0, 3), (3, 6), (6, 8)]

    with tile.TileContext(nc) as tc, ExitStack() as ctx:
        const = ctx.enter_context(tc.tile_pool(name="const", bufs=1))
        zpool = ctx.enter_context(tc.tile_pool(name="zp", bufs=2, space="PSUM"))
        lpool = ctx.enter_context(tc.tile_pool(name="lp", bufs=2, space="PSUM"))
        opool = ctx.enter_context(tc.tile_pool(name="op", bufs=2))

        # --- resident tiles + input DMA ---
        # h history for all chunks in ONE tile: chunk k lives at columns
        # [k*HL, (k+1)*HL) so a single ACT can write several chunks via a
        # strided 3-D AP.
        h_big = const.tile([128, KC * HL], bf16, tag="hb", name="h_big")
        hv = h_big.rearrange("p (j c) -> p j c", j=KC)
        for k in range(KC):
            nc.sync.dma_start(
                out=hv[:, k, 0:BC], in_=h0_d[k * 128:(k + 1) * 128, :]
            )
        wh_sb = [const.tile([128, H], bf16, tag=f"wh{k}", name=f"wh{k}") for k in range(KC)]
        for k in range(KC):
            nc.sync.dma_start(out=wh_sb[k][:], in_=wh_d[k])
        ew_sb = const.tile([VP, H], bf16, tag="ew", name="ew_sb")
        nc.sync.dma_start(out=ew_sb[:], in_=ew_d[:])
        oh_sb = const.tile([VP, cols], bf16, tag="oh", name="oh_sb")
        nc.sync.dma_start(out=oh_sb[:], in_=oh_d[:])
        fw_sb = [const.tile([128, V], bf16, tag=f"fw{k}", name=f"fw{k}") for k in range(KC)]
        for k in range(KC):
            nc.sync.dma_start(out=fw_sb[k][:], in_=fw_d[k])
        fb_sb = const.tile([V, 1], f32, tag="fb", name="fb_sb")
        nc.sync.dma_start(out=fb_sb[:], in_=fb_d[:])
        hf_sb = const.tile([128, KC * BC], f32, tag="hf", name="hf_sb")
        hfv = hf_sb.rearrange("p (j c) -> p j c", j=KC)

        # --- the scan ---
        for t in range(n_steps):
            rd = slice(t * BC, (t + 1) * BC)
            wr = slice((t + 1) * BC, (t + 2) * BC)
            for lo, hi in SPLITS:
                nj = hi - lo
                ps = zpool.tile([128, nj * BC], f32, tag=f"z{lo}",
                                name=f"z{lo}_ps")
                for j in range(lo, hi):
                    js = slice(j * 128, (j + 1) * 128)
                    po = ps[:, (j - lo) * BC:(j - lo + 1) * BC]
                    nc.tensor.matmul(
                        po, lhsT=ew_sb[:, js], rhs=oh_sb[:, rd],
                        start=True, stop=False,
                    )
                    for k in range(KC):
                        nc.tensor.matmul(
                            po, lhsT=wh_sb[k][:, js], rhs=hv[:, k, rd],
                            start=False, stop=(k == KC - 1),
                        )
                pv = ps.rearrange("p (j c) -> p j c", j=nj)
                nc.scalar.activation(hv[:, lo:hi, wr], pv[:], Tanh)
                if t == n_steps - 1:
                    nc.scalar.activation(hfv[:, lo:hi, :], pv[:], Tanh)

        for k in range(KC):
            nc.gpsimd.dma_start(
                out=hf_d[k * 128:(k + 1) * 128, :], in_=hfv[:, k, :],
            )

        # --- logits GEMM: logits.T[v, i] over history (steps 1..n) ---
        n_sl = (cols + 511) // 512
        for s in range(n_sl):
            c0 = s * 512
            cw = min(512, cols - c0)
            ps = lpool.tile([V, 512], f32, tag="lg", name="lg_ps")
            for k in range(KC):
                nc.tensor.matmul(
                    ps[:, :cw], lhsT=fw_sb[k][:],
                    rhs=hv[:, k, BC + c0: BC + c0 + cw],
                    start=(k == 0), stop=(k == KC - 1),
                )
            ot = opool.tile([V, 512], f32, tag="o", name="lg_sb")
            nc.scalar.activation(ot[:, :cw], ps[:, :cw], Ident, bias=fb_sb[:])
            nc.gpsimd.dma_start(out=lg_d[:, c0:c0 + cw], in_=ot[:, :cw])

    nc.compile()
    return nc


def _get_nc(n_steps=L):
    if n_steps not in _nc_cache:
        _nc_cache[n_steps] = _build_nc(n_steps)
    return _nc_cache[n_steps]


def _prep_shared(embedding, W_e, W_h, fc_w, fc_b):
    embW = embedding.astype(np.float32) @ W_e.astype(np.float32)
    ew = np.zeros((VP, H), _BF16)
    ew[:V] = embW.astype(_BF16)
    wh = np.ascontiguousarray(W_h.astype(_BF16).reshape(KC, 128, H))
    fw = np.ascontiguousarray(
        fc_w.T.astype(_BF16).reshape(KC, 128, V)
    )
    fb = np.ascontiguousarray(fc_b.astype(np.float32).reshape(V, 1))
    return ew, wh, fw, fb


def kernel(x, hidden, embedding, W_e, W_h, fc_w, fc_b):
    from concourse.bass_utils import run_bass_kernel_spmd

    x = np.asarray(x)
    hidden = np.asarray(hidden, dtype=np.float32)
    ew, wh, fw, fb = _prep_shared(
        np.asarray(embedding), np.asarray(W_e), np.asarray(W_h),
        np.asarray(fc_w), np.asarray(fc_b),
    )

    in_maps = []
    for c in range(NCORES):
        xc = x[c * BC:(c + 1) * BC].astype(np.int64)
        xi = xc.T.reshape(-1)                 # column i = t*BC + b
        oh = np.zeros((VP, LB), _BF16)
        oh[xi, np.arange(LB)] = 1
        h0 = np.ascontiguousarray(hidden[c * BC:(c + 1) * BC].T.astype(_BF16))
        in_maps.append(
            dict(oh=oh, h0=h0, wh=wh, ew=ew, fw=fw, fb=fb)
        )

    nc = _get_nc(L)
    out = run_bass_kernel_spmd(nc, in_maps, core_ids=list(range(NCORES)))
    LAST_RUN["results"] = out
    res = out.results

    logits = np.empty((B, L, V), np.float32)
    final_h = np.empty((B, H), np.float32)
    for c in range(NCORES):
        lg = np.asarray(res[c]["lg"], dtype=np.float32)
        logits[c * BC:(c + 1) * BC] = lg.reshape(V, L, BC).transpose(2, 1, 0)
        final_h[c * BC:(c + 1) * BC] = np.asarray(
            res[c]["hf"], dtype=np.float32
        ).T
    return logits, final_h
